# revision 3
# baseline (speedup 1.0000x reference)
"""CycleVAR VQ-codebook encoder kernel for Trainium2 (8 NeuronCores).

Contract: kernel(**inputs) takes FULL inputs
  f_src      [128, 32, 16, 16] fp32
  emb_weight [4096, 32] fp32
and returns the FULL output x_var [128, 340, 32] fp32.

The reference's x_var depends only on stages pn in (1, 2, 4, 8); the pn=16
stage is dead code.  The straight-through output equals the *hard* quantized
embedding, so only the argmin-distance code index matters per token.

Sharding: data-parallel over batch (16 images per core), codebook replicated.

Per-core pipeline (B=16, C=32, H=W=16, S=256), stage si with pn in (1,2,4,8),
P=pn^2, ntok=16P, tokens t=b*P+p:
  z-down  (PE):  psz_g[(b4,c), p] = sum_s f_rest[s, bc] A_pn[p, s], one matmul
                 pair (2 s-chunks) per group of 4 images.
  zaug    (ACT): partition-fold psz groups -> zaug[0:32, b*P:(b+1)*P]; row 32
                 is constant 1.0.  Replicated to partition base 64 by DMA.
  scores  (PE):  per 128-token block, 2 PSUM chunks of 2048 codes; each chunk
                 is two fp32 matmuls per PE row-quadrant (codes host-permuted
                 so chunk columns are contiguous in scan order).
                 K=33: rows 0..31 = z, row 32 * (-0.5|e|^2) folds the bias.
  argmax  (DVE): grouped reduce off PSUM -> gmax[t, 32] (groups of 128);
                 max8 + max_index -> winning group g*.
          (ACT): chunk copy PSUM -> scsb; one DMA per chunk -> scd DRAM in
                 [t, g, j] row layout (coalesced; queue-parallel).
          (DMA): indirect gather row t*32+g* -> grp[t, 128]; max_index ->
                 j*; vidx = 128 g* + j* (scan order).
  h       (DMA): gpsimd indirect gather from the host-permuted codebook.
  up      (PE):  h_up = U_pn @ h_sp (2 row-quadrants x 2 column halves).
  update  (DVE): f_rest -= h_up (si<3); f_partial += h_up.
  x_out   (PE):  x_si = A_pn' @ f_partial (si<3); si=3 emits f_partial.
Output DRAM x_out [340, (b,c)]; host transposes to [b, 340, c].
"""

import os

import numpy as np

import concourse.bacc as bacc
import concourse.bass as bass
import concourse.mybir as mybir
import concourse.tile as tile
from concourse.bass import IndirectOffsetOnAxis
from concourse.bass_utils import run_bass_kernel_spmd

N_CORES = 8
B_FULL = 128
B_LOC = B_FULL // N_CORES  # 16
C = 32
H = 16
S = H * H  # 256
V = 4096
PNS = (1, 2, 4, 8)
ROW_OFF = (0, 4, 20, 84)  # x_var row offsets per stage
NTOK_OUT = 340

F32 = mybir.dt.float32
U32 = mybir.dt.uint32
AX = mybir.AxisListType
ALU = mybir.AluOpType
ACTF = mybir.ActivationFunctionType

LAST_RESULTS = None  # test harness introspection


def _keys_cubic(x, a=-0.5):
    x = np.abs(x)
    return np.where(
        x <= 1,
        (a + 2) * x**3 - (a + 3) * x**2 + 1,
        np.where(x < 2, a * x**3 - 5 * a * x**2 + 8 * a * x - 4 * a, 0.0),
    )


def _resize_matrix_1d(n_in, n_out):
    # matches jax.image.resize(method='cubic') for upsampling
    scale = n_out / n_in
    U = np.zeros((n_out, n_in), np.float64)
    for i in range(n_out):
        x = (i + 0.5) / scale - 0.5
        w = _keys_cubic(x - np.arange(n_in))
        s = w.sum()
        if s != 0:
            w = w / s
        U[i] = w
    return U


def _up_matrix(pn):
    # [S, pn*pn] bicubic upsample matrix (kron of separable 1D)
    if pn == H:
        return np.eye(S, dtype=np.float32)
    U1 = _resize_matrix_1d(pn, H)
    return np.kron(U1, U1).astype(np.float32)


def _down_matrix(pn):
    # [pn*pn, S] exact area mean (r = H//pn, weight 1/r^2, exact pow2)
    r = H // pn
    A = np.zeros((pn * pn, S), np.float32)
    w = np.float32(1.0 / (r * r))
    for pi in range(pn):
        for pj in range(pn):
            for di in range(r):
                for dj in range(r):
                    A[pi * pn + pj, (pi * r + di) * H + (pj * r + dj)] = w
    return A


def _scan_perm():
    # scan position i = 2048*c + j maps to code: quadrant 0 streams codes
    # [1024c, 1024c+1024) into columns 0:1024 of chunk c; quadrant 1 streams
    # codes [2048+1024c, ...) into columns 1024:2048.
    perm = np.zeros(V, np.int64)
    for c in range(2):
        for j in range(2048):
            perm[2048 * c + j] = (
                1024 * c + j if j < 1024 else 2048 + 1024 * c + (j - 1024)
            )
    return perm


def _build_program():
    nc = bacc.Bacc(trn_type="TRN2", target_bir_lowering=False, debug=False)

    # DRAM I/O (per core)
    f_in = nc.dram_tensor("f_pre", [2, 128, 512], F32, kind="ExternalInput").ap()
    eaug_in = nc.dram_tensor("eaug", [33, V], F32, kind="ExternalInput").ap()
    emb_in = nc.dram_tensor("embt", [V, C], F32, kind="ExternalInput").ap()
    a_in = {
        pn: nc.dram_tensor(f"a{pn}", [2, 128, pn * pn], F32, kind="ExternalInput").ap()
        for pn in PNS
    }
    u_in = {
        pn: nc.dram_tensor(f"u{pn}", [pn * pn, 256], F32, kind="ExternalInput").ap()
        for pn in PNS
    }
    x_out = nc.dram_tensor("xout", [NTOK_OUT, 512], F32, kind="ExternalOutput").ap()

    with tile.TileContext(nc) as tc:
        from contextlib import ExitStack

        ctx = ExitStack()
        const = ctx.enter_context(tc.tile_pool(name="const", bufs=1))
        work = ctx.enter_context(tc.tile_pool(name="work", bufs=2))
        small = ctx.enter_context(tc.tile_pool(name="small", bufs=3))
        psum = ctx.enter_context(tc.tile_pool(name="psum", bufs=2, space="PSUM"))
        dram = ctx.enter_context(tc.tile_pool(name="dram", bufs=2, space="DRAM"))

        # ---- constants to SBUF ----
        f_rest = [const.tile([128, 512], F32, name=f"frest{ch}") for ch in range(2)]
        f_partial = [const.tile([128, 512], F32, name=f"fpart{ch}") for ch in range(2)]
        a_sb = {}
        for pn in PNS:
            P = pn * pn
            a_sb[pn] = [const.tile([128, P], F32, name=f"a{pn}_{ch}") for ch in range(2)]
        for ch in range(2):
            nc.sync.dma_start(f_rest[ch], f_in[ch])
        for ch in range(2):
            nc.scalar.dma_start(a_sb[1][ch], a_in[1][ch])
        eaug_big = const.tile([97, V], F32)
        nc.sync.dma_start(eaug_big[0:33], eaug_in)
        nc.scalar.dma_start(eaug_big[64:97], eaug_in)
        for pn in PNS[1:]:
            for ch in range(2):
                nc.sync.dma_start(a_sb[pn][ch], a_in[pn][ch])
        u_sb = {}
        for pn in PNS:
            P = pn * pn
            u_sb[pn] = const.tile([128, 256], F32, name=f"u{pn}")
            nc.scalar.dma_start(u_sb[pn][0:P], u_in[pn])
            nc.sync.dma_start(u_sb[pn][64 : 64 + P], u_in[pn])
        for ch in range(2):
            nc.vector.memset(f_partial[ch], 0.0)

        zaug_big = const.tile([97, 1024], F32)
        nc.vector.memset(zaug_big[32:33, :], 1.0)

        toff32 = const.tile([128, 1], U32)  # t*32 per partition
        nc.gpsimd.iota(toff32, pattern=[[1, 1]], base=0, channel_multiplier=32)

        for si, pn in enumerate(PNS):
            P = pn * pn
            ntok = B_LOC * P
            nblk = (ntok + 127) // 128
            zaug = zaug_big[0:33]

            # ---- z-down: 4 image groups x 2 s-chunks ----
            for g in range(4):
                psz = psum.tile([128, 2048], F32, tag="pq")
                for ch in range(2):
                    nc.tensor.matmul(
                        psz[:, :P],
                        f_rest[ch][:, 128 * g : 128 * (g + 1)],
                        a_sb[pn][ch][:, :P],
                        start=(ch == 0),
                        stop=(ch == 1),
                    )
                for j in range(4):
                    b = 4 * g + j
                    nc.scalar.activation(
                        zaug[0:32, b * P : (b + 1) * P],
                        psz[32 * j : 32 * (j + 1), :P],
                        ACTF.Copy,
                    )

            h_sp = work.tile([128, 512], F32, tag="hsp", name=f"hsp{si}")

            for blk in range(nblk):
                t0 = blk * 128
                tw = min(128, ntok - t0)

                # replicate this block's tokens (and ones row) to base 64
                nc.sync.dma_start(
                    zaug_big[64:97, t0 : t0 + tw], zaug_big[0:33, t0 : t0 + tw]
                )

                scsb = work.tile([128, V], F32, tag="scsb", bufs=2)
                gmax = small.tile([128, 32], F32, tag="gmax")
                scd = dram.tile([128 * 32, 128], F32, tag="scd")
                for c in range(2):
                    pq = psum.tile([128, 2048], F32, tag="pq")
                    for q in range(2):
                        nc.tensor.matmul(
                            pq[:tw, 512 * q : 512 * (q + 1)],
                            zaug_big[0:33, t0 : t0 + tw],
                            eaug_big[0:33, 1024 * c + 512 * q : 1024 * c + 512 * (q + 1)],
                            start=True,
                            stop=True,
                            tile_position=(0, 0),
                        )
                        nc.tensor.matmul(
                            pq[:tw, 1024 + 512 * q : 1024 + 512 * (q + 1)],
                            zaug_big[64:97, t0 : t0 + tw],
                            eaug_big[
                                64:97,
                                2048 + 1024 * c + 512 * q : 2048 + 1024 * c + 512 * (q + 1),
                            ],
                            start=True,
                            stop=True,
                            tile_position=(64, 0),
                        )
                    nc.vector.reduce_max(
                        gmax[:tw, 16 * c : 16 * (c + 1)],
                        pq[:tw].rearrange("t (g j) -> t g j", g=16),
                        axis=AX.X,
                    )
                    nc.scalar.activation(
                        scsb[:tw, 2048 * c : 2048 * (c + 1)], pq[:tw], ACTF.Copy
                    )
                    nc.sync.dma_start(
                        scd.rearrange("(t g) j -> t g j", g=32)[
                            :tw, 16 * c : 16 * (c + 1)
                        ],
                        scsb[:tw, 2048 * c : 2048 * (c + 1)].rearrange(
                            "t (g j) -> t g j", g=16
                        ),
                    )

                top8 = small.tile([128, 8], F32, tag="top8")
                gidx = small.tile([128, 8], U32, tag="gidx")
                nc.vector.max(top8[:tw], gmax[:tw])
                nc.vector.max_index(gidx[:tw], top8[:tw], gmax[:tw])

                off = small.tile([128, 1], U32, tag="off")
                nc.vector.tensor_tensor(
                    off[:tw], toff32[:tw], gidx[:tw, 0:1], op=ALU.add
                )
                grp = small.tile([128, 128], F32, tag="grp")
                nc.gpsimd.indirect_dma_start(
                    grp[:tw], None, scd[:, :], IndirectOffsetOnAxis(ap=off[:tw], axis=0)
                )
                jidx = small.tile([128, 8], U32, tag="jidx")
                nc.vector.max_index(jidx[:tw], top8[:tw], grp[:tw])
                vidx = small.tile([128, 1], U32, tag="vidx")
                nc.vector.tensor_scalar_mul(vidx[:tw], gidx[:tw, 0:1], 128)
                nc.vector.tensor_tensor(
                    vidx[:tw], vidx[:tw], jidx[:tw, 0:1], op=ALU.add
                )

                htok = small.tile([128, C], F32, tag="htok")
                nc.gpsimd.indirect_dma_start(
                    htok[:tw],
                    None,
                    emb_in,
                    IndirectOffsetOnAxis(ap=vidx[:tw], axis=0),
                )
                # scatter into h_sp [p, (b,c)]: per-b partition-shift copies
                qs = [nc.sync, nc.scalar]
                for j, b in enumerate(range(t0 // P, (t0 + tw) // P)):
                    r0 = b * P - t0
                    qs[j % 2].dma_start(
                        h_sp[0 : min(P, tw), 32 * b : 32 * (b + 1)],
                        htok[r0 : r0 + min(P, tw)],
                    )

            # ---- bicubic up + residual/partial updates ----
            nc.scalar.dma_start(h_sp[64 : 64 + P], h_sp[0:P])
            for hf in range(2):
                cs = slice(256 * hf, 256 * (hf + 1))
                pus = []
                for ch in range(2):
                    pu = psum.tile([128, 2048], F32, tag="pq")
                    nc.tensor.matmul(
                        pu[:, 0:256],
                        u_sb[pn][64 * ch : 64 * ch + P, 128 * ch : 128 * (ch + 1)],
                        h_sp[64 * ch : 64 * ch + P, cs],
                        start=True,
                        stop=True,
                        tile_position=(64 * ch, 0),
                    )
                    pus.append(pu)
                for ch in range(2):
                    pu = pus[ch]
                    if si < 3:
                        nc.vector.tensor_tensor(
                            f_rest[ch][:, cs], f_rest[ch][:, cs], pu[:, 0:256],
                            op=ALU.subtract,
                        )
                    nc.vector.tensor_tensor(
                        f_partial[ch][:, cs], f_partial[ch][:, cs], pu[:, 0:256],
                        op=ALU.add,
                    )

            # ---- x output ----
            if si < 3:
                pn2 = PNS[si + 1]
                P2 = pn2 * pn2
                px = psum.tile([128, 2048], F32, tag="pq")
                for ch in range(2):
                    nc.tensor.matmul(
                        px[:P2, 0:512],
                        a_sb[pn2][ch][:, :P2],
                        f_partial[ch],
                        start=(ch == 0),
                        stop=(ch == 1),
                    )
                x_sb = small.tile([max(P2, 1), 512], F32, tag="xsb")
                nc.scalar.activation(x_sb[:P2], px[:P2, 0:512], ACTF.Copy)
                nc.sync.dma_start(x_out[ROW_OFF[si] : ROW_OFF[si] + P2], x_sb[:P2])
            else:
                for ch in range(2):
                    for hf in range(2):
                        cs = slice(256 * hf, 256 * (hf + 1))
                        qs2 = [nc.sync, nc.scalar]
                        qs2[hf].dma_start(
                            x_out[84 + 128 * ch : 84 + 128 * (ch + 1), cs],
                            f_partial[ch][:, cs],
                        )

        ctx.close()

    nc.compile()
    return nc


_PROGRAM = None


def _get_program():
    global _PROGRAM
    if _PROGRAM is None:
        _PROGRAM = _build_program()
    return _PROGRAM


def kernel(f_src, emb_weight):
    global LAST_RESULTS
    f_src = np.asarray(f_src, dtype=np.float32)
    emb_weight = np.asarray(emb_weight, dtype=np.float32)

    e64 = emb_weight.astype(np.float64)
    eaug = np.concatenate(
        [emb_weight.T, (-0.5 * (e64 * e64).sum(1)).astype(np.float32)[None, :]], axis=0
    )  # [33, V]
    emb_perm = np.ascontiguousarray(emb_weight[_scan_perm()])  # [V, C]

    a_mats = {}
    u_mats = {}
    for pn in PNS:
        P = pn * pn
        a_mats[pn] = np.ascontiguousarray(
            _down_matrix(pn).T.reshape(2, 128, P)
        )  # [2, 128, P]
        u_mats[pn] = np.ascontiguousarray(_up_matrix(pn).T)  # [P, 256]

    in_maps = []
    for core in range(N_CORES):
        fb = f_src[core * B_LOC : (core + 1) * B_LOC]  # [16, 32, 16, 16]
        f_pre = (
            fb.reshape(B_LOC, C, S).transpose(2, 0, 1).reshape(2, 128, 512)
        )  # [s, b, c]
        m = {
            "f_pre": np.ascontiguousarray(f_pre),
            "eaug": np.ascontiguousarray(eaug),
            "embt": emb_perm,
        }
        for pn in PNS:
            m[f"a{pn}"] = a_mats[pn]
            m[f"u{pn}"] = u_mats[pn]
        in_maps.append(m)

    nc = _get_program()
    trace = bool(os.environ.get("CVAR_TRACE"))
    try:
        res = run_bass_kernel_spmd(
            nc,
            in_maps,
            core_ids=list(range(N_CORES)),
            trace=trace,
        )
    except ModuleNotFoundError:
        res = run_bass_kernel_spmd(
            nc, in_maps, core_ids=list(range(N_CORES)), trace=False
        )
    LAST_RESULTS = res

    outs = []
    for core in range(N_CORES):
        xo = res.results[core]["xout"]  # [340, 512]
        outs.append(xo.reshape(NTOK_OUT, B_LOC, C).transpose(1, 0, 2))
    return np.ascontiguousarray(np.concatenate(outs, axis=0))
